# revision 8
# baseline (speedup 1.0000x reference)
"""Trainium2 Bass kernel for nn_NeuralNet_19516331393457 (dense_mlp).

Pipeline: x = embed[data] (48-entry table); h1 = relu(x@W1+b1);
h2 = tanh(h1@W2+b2); out = h2@W3+b3; return out[argmax(F(out0, out1))].

v2 strategy (data-parallel over N=500000 on 8 cores, fp16 device math):
  - Host: tiny-table gather embed[data] in fp16, tile-blocked transpose to
    [NQ, 128, 2048] per core (quads of 4 chunks x 512 samples).
  - Device per quad: MM1 x4 (fp16, W1 stationary) -> 2x [128,1024] PSUM;
    relu eviction split DVE/ACT -> h1 fp16; MM2 x4 column-packed 2-up
    (tile_position col 0/64) into one [128,1024] PSUM; one ACT tanh evicts
    4 chunks; MM3 x4 quad-packed via tile_position (row 0/64 x col
    0/32/64/96) into sparse partitions of the just-freed p1a tile (saves
    PSUM banks); one DVE copy -> fp16 staging; strided output DMAs.
  - Host: decode [4, NQ*512] fp16 outs, F in fp64, exact top-K rescore in
    fp32 (device ordering only needs winner in top-K; fp16 keeps it rank 1).
"""

import numpy as np
import ml_dtypes

import concourse.mybir as mybir
import concourse.tile as tile
from concourse import bacc
from concourse.bass_utils import run_bass_kernel_spmd

N = 500000
D = 128
H1 = 128
H2 = 64
NCLS = 2
NCORES = 8
CHUNK = 512
NPC_RAW = N // NCORES              # 62500 samples per core
NQ = 31                            # quads per core (4 chunks each)
CHUNKS = 4 * NQ                    # 124
NPC = CHUNKS * CHUNK               # 63488 padded samples per core
PIECE = 8                          # quads per output staging piece
NPIECE = -(-NQ // PIECE)           # 4

_F32 = mybir.dt.float32
_F16 = mybir.dt.float16
_BF16 = mybir.dt.bfloat16


def _build_bass():
    nc = bacc.Bacc(
        "TRN2",
        target_bir_lowering=False,
        debug=False,
        enable_asserts=False,
        num_devices=NCORES,
    )
    x_t = nc.dram_tensor("x_t", [NQ, D, 4 * CHUNK], _BF16, kind="ExternalInput")
    w1 = nc.dram_tensor("w1", [D, H1], _BF16, kind="ExternalInput")
    w2 = nc.dram_tensor("w2", [H1, H2], _BF16, kind="ExternalInput")
    # W3 duplicated on rows 0:64 and 64:128 (for tile_position row 0/64)
    w3d = nc.dram_tensor("w3d", [128, NCLS], _BF16, kind="ExternalInput")
    b1 = nc.dram_tensor("b1", [H1, 1], _F32, kind="ExternalInput")
    # b2 duplicated: [b2; b2] for the 2-up partition-packed tanh
    b2d = nc.dram_tensor("b2d", [128, 1], _F32, kind="ExternalInput")
    # class c of chunk 4q+j, sample i -> out_c[j, q*512 + i]
    out0 = nc.dram_tensor("out0", [4, NQ * CHUNK], _F16, kind="ExternalOutput")
    out1 = nc.dram_tensor("out1", [4, NQ * CHUNK], _F16, kind="ExternalOutput")

    with tile.TileContext(nc) as tc:
        with (
            tc.tile_pool(name="w", bufs=1) as wpool,
            tc.tile_pool(name="x", bufs=3) as xpool,
            tc.tile_pool(name="h1", bufs=4) as h1pool,
            tc.tile_pool(name="h2", bufs=3) as h2pool,
            tc.tile_pool(name="ob", bufs=2) as obpool,
            tc.tile_pool(name="p1", bufs=2, space="PSUM") as p1pool,
            tc.tile_pool(name="p2", bufs=1, space="PSUM") as p2pool,
            tc.tile_pool(name="po", bufs=2, space="PSUM") as popool,
        ):
            w1sb = wpool.tile([D, H1], _BF16)
            nc.sync.dma_start(w1sb[:], w1[:, :])
            w2sb = wpool.tile([H1, H2], _BF16)
            nc.sync.dma_start(w2sb[:], w2[:, :])
            w3sb = wpool.tile([128, NCLS], _BF16)
            nc.sync.dma_start(w3sb[:], w3d[:, :])
            b1sb = wpool.tile([H1, 1], _F32)
            nc.sync.dma_start(b1sb[:], b1[:, :])
            b2sb = wpool.tile([128, 1], _F32)
            nc.sync.dma_start(b2sb[:], b2d[:, :])

            obs = {}
            for q in range(NQ):
                xt = xpool.tile([D, 4 * CHUNK], _BF16, name=f"xt{q}", tag="xt")
                nc.sync.dma_start(xt[:], x_t[q, :, :])

                # MM1 x4 -> two [128,1024] psum tiles
                p1a = p1pool.tile([H1, 2 * CHUNK], _F32, name=f"p1a{q}",
                                  tag="p1")
                p1b = p1pool.tile([H1, 2 * CHUNK], _F32, name=f"p1b{q}",
                                  tag="p1")
                for h, p1t in ((0, p1a), (1, p1b)):
                    for s in range(2):
                        c = 2 * h + s
                        nc.tensor.matmul(
                            p1t[:, s * CHUNK : (s + 1) * CHUNK],
                            w1sb[:],
                            xt[:, c * CHUNK : (c + 1) * CHUNK],
                            start=True, stop=True,
                        )

                # relu evictions: h1 = max(p1 + b1, 0), fp16; DVE + ACT
                h1a = h1pool.tile([H1, 2 * CHUNK], _BF16, name=f"h1a{q}",
                                  tag="h1")
                nc.vector.tensor_scalar(
                    h1a[:], p1a[:], b1sb[:], 0.0,
                    mybir.AluOpType.add, mybir.AluOpType.max,
                )
                h1b = h1pool.tile([H1, 2 * CHUNK], _BF16, name=f"h1b{q}",
                                  tag="h1")
                nc.scalar.activation(
                    h1b[:], p1b[:], mybir.ActivationFunctionType.Relu,
                    bias=b1sb[:],
                )

                # MM2 x4 column-packed 2-up into one [128,1024] psum tile
                p2 = p2pool.tile([128, 2 * CHUNK], _F32, name=f"p2_{q}",
                                 tag="p2")
                for h, h1t in ((0, h1a), (1, h1b)):
                    for s in range(2):
                        nc.tensor.matmul(
                            p2[s * H2 : (s + 1) * H2,
                               h * CHUNK : (h + 1) * CHUNK],
                            w2sb[:],
                            h1t[:, s * CHUNK : (s + 1) * CHUNK],
                            start=True, stop=True,
                        )

                # one ACT tanh evicts all 4 chunks: h2 fp16 [128, 1024]
                h2t = h2pool.tile([128, 2 * CHUNK], _BF16, name=f"h2_{q}",
                                  tag="h2")
                nc.scalar.activation(
                    h2t[:], p2[:], mybir.ActivationFunctionType.Tanh,
                    bias=b2sb[:],
                )

                # MM3 x4 quad-packed into sparse partitions {32j, 32j+1} of
                # a [128,512] psum tile; rhs = h2 rows [64s:64s+64].
                po = popool.tile([128, CHUNK], _F32, name=f"po{q}", tag="po")
                for h in range(2):
                    for s in range(2):
                        j = 2 * h + s
                        nc.tensor.matmul(
                            po[32 * j : 32 * j + NCLS, :],
                            w3sb[64 * s : 64 * s + 64, :],
                            h2t[64 * s : 64 * s + 64,
                                h * CHUNK : (h + 1) * CHUNK],
                            start=True, stop=True,
                            tile_position=(64 * s, 32 * j),
                        )

                # one DVE copy evicts the 4 chunk-outputs (rows 0:98 dense;
                # rows between the written pairs carry harmless relu'd MM1
                # values, never DMA'd out)
                piece = q // PIECE
                if piece not in obs:
                    obs[piece] = obpool.tile(
                        [128, PIECE * CHUNK], _F16, name=f"ob{piece}",
                        tag="ob",
                    )
                qq = q % PIECE
                nc.vector.tensor_copy(
                    obs[piece][0:98, qq * CHUNK : (qq + 1) * CHUNK],
                    po[0:98, :],
                )

                if q == NQ - 1 or qq == PIECE - 1:
                    cols = (qq + 1) * CHUNK
                    base = piece * PIECE * CHUNK
                    nc.sync.dma_start(
                        out0[:, base : base + cols],
                        obs[piece][0:97:32, 0:cols],
                    )
                    nc.sync.dma_start(
                        out1[:, base : base + cols],
                        obs[piece][1:98:32, 0:cols],
                    )

    nc.compile()
    return nc


_NC_CACHE = None


def _get_nc():
    global _NC_CACHE
    if _NC_CACHE is None:
        _NC_CACHE = _build_bass()
    return _NC_CACHE


def _F64(x, y):
    return (
        3.0 * (1.0 - x) ** 2 * np.exp(-(x**2) - (y + 1.0) ** 2)
        - 10.0 * (x / 5.0 - x**3 - y**5) * np.exp(-(x**2) - y**2)
        - 1.0 / (3.0 ** np.exp(-((x + 1.0) ** 2) - y**2))
    )


def make_in_maps(data, embed, W1, b1, W2, b2, W3, b3):
    data = np.asarray(data)
    table16 = np.asarray(embed, dtype=np.float32).reshape(-1).astype(ml_dtypes.bfloat16)
    W1c = np.ascontiguousarray(np.asarray(W1, np.float32).astype(ml_dtypes.bfloat16))
    W2c = np.ascontiguousarray(np.asarray(W2, np.float32).astype(ml_dtypes.bfloat16))
    W3c = np.asarray(W3, np.float32).astype(ml_dtypes.bfloat16)
    w3d = np.concatenate([W3c, W3c], axis=0)          # [128, 2]
    b1c = np.ascontiguousarray(b1, dtype=np.float32).reshape(H1, 1)
    b2c = np.asarray(b2, dtype=np.float32).reshape(H2, 1)
    b2d = np.concatenate([b2c, b2c], axis=0)          # [128, 1]

    in_maps = []
    for c in range(NCORES):
        dshard = data[c * NPC_RAW : (c + 1) * NPC_RAW]
        dpad = np.zeros((NPC, D), dtype=dshard.dtype)
        dpad[:NPC_RAW] = dshard
        # fused gather + tile-blocked transpose: [NQ, D(feat), 2048]
        xt = np.ascontiguousarray(
            table16[dpad.reshape(NQ, 4 * CHUNK, D).transpose(0, 2, 1)]
        )
        in_maps.append(
            {"x_t": xt, "w1": W1c, "w2": W2c, "w3d": w3d,
             "b1": b1c, "b2d": b2d}
        )
    return in_maps


def _decode_outs(res):
    """-> out0_all, out1_all fp32 arrays of shape [N] (padding stripped)."""
    o0s, o1s = [], []
    for c in range(NCORES):
        o0 = np.asarray(res.results[c]["out0"], np.float32).reshape(4, NQ, CHUNK)
        o1 = np.asarray(res.results[c]["out1"], np.float32).reshape(4, NQ, CHUNK)
        o0s.append(o0.transpose(1, 0, 2).reshape(-1)[:NPC_RAW])
        o1s.append(o1.transpose(1, 0, 2).reshape(-1)[:NPC_RAW])
    return np.concatenate(o0s), np.concatenate(o1s)


def kernel(data, embed, W1, b1, W2, b2, W3, b3):
    data = np.asarray(data)
    nc = _get_nc()
    in_maps = make_in_maps(data, embed, W1, b1, W2, b2, W3, b3)
    res = run_bass_kernel_spmd(nc, in_maps, core_ids=list(range(NCORES)))
    o0, o1 = _decode_outs(res)

    # device ordering (fp16 keeps the true winner at rank 1; the exact
    # fp32 rescore below only needs it inside the top-K shortlist)
    pred = _F64(o0.astype(np.float64), o1.astype(np.float64))
    K = 4096
    cand = np.argpartition(pred, N - K)[N - K:]

    table32 = np.asarray(embed, dtype=np.float32).reshape(-1)
    W1f = np.asarray(W1, np.float32)
    W2f = np.asarray(W2, np.float32)
    W3f = np.asarray(W3, np.float32)
    xk = table32[data[cand]]
    hk = np.maximum(xk @ W1f + np.asarray(b1, np.float32), 0.0)
    hk = np.tanh(hk @ W2f + np.asarray(b2, np.float32))
    ok = hk @ W3f + np.asarray(b3, np.float32)
    pk = _F64(ok[:, 0].astype(np.float64), ok[:, 1].astype(np.float64))
    return ok[int(np.argmax(pk))].astype(np.float32)


# revision 9
# speedup vs baseline: 1.1388x; 1.1388x over previous
"""Trainium2 Bass kernel for nn_NeuralNet_19516331393457 (dense_mlp).

Pipeline: x = embed[data] (48-entry table); h1 = relu(x@W1+b1);
h2 = tanh(h1@W2+b2); out = h2@W3+b3; return out[argmax(F(out0, out1))].

v3 strategy (data-parallel over N=500000 on 8 cores, bf16 device math):
  - Host: tiny-table gather embed[data] in bf16, tile-blocked transpose to
    [NQ, 128, 2048] per core (quads of 4 chunks x 512 samples).
  - Device, software-pipelined per quad q: MM1(q) x4 -> 2x [128,1024] PSUM;
    relu(q) split DVE/ACT -> h1 bf16; then the *previous* quad's tail:
    MM2(q-1) x4 column-packed 2-up into one [128,1024] PSUM; one ACT tanh
    evicts 4 chunks; MM3 via block-diag W3blk=[W3 0; 0 W3] -> dense [4,512]
    outputs at partition 32p of a per-2-quad po bank; one DVE cast per 2
    quads -> fp16 staging; strided output DMAs per 8-quad piece.
  - Host: decode outs, F in fp64, exact top-K rescore in fp32 (device
    ordering only needs the winner inside the top-K shortlist).
"""

import numpy as np
import ml_dtypes

import concourse.mybir as mybir
import concourse.tile as tile
from concourse import bacc
from concourse.bass_utils import run_bass_kernel_spmd

N = 500000
D = 128
H1 = 128
H2 = 64
NCLS = 2
NCORES = 8
CHUNK = 512
NPC_RAW = N // NCORES              # 62500 samples per core
NQ = 31                            # quads per core (4 chunks each)
CHUNKS = 4 * NQ                    # 124
NPC = CHUNKS * CHUNK               # 63488 padded samples per core
NG = 16                            # 2-quad groups (last is half)
PIECE = 8                          # quads per output staging piece
NPIECE = 4

_F32 = mybir.dt.float32
_F16 = mybir.dt.float16
_BF16 = mybir.dt.bfloat16


def _quad_head(nc, q, pools, tls):
    """Issue DMA + MM1 + relu for quad q."""
    (xpool, h1pool, h2pool, obs_pool, p1pool, p2pool, popool) = pools
    xt = xpool.tile([D, 4 * CHUNK], _BF16, name=f"xt{q}", tag="xt")
    nc.sync.dma_start(xt[:], tls["x_t"][q, :, :])

    p1a = p1pool.tile([H1, 2 * CHUNK], _F32, name=f"p1a{q}", tag="p1")
    p1b = p1pool.tile([H1, 2 * CHUNK], _F32, name=f"p1b{q}", tag="p1")
    for h, p1t in ((0, p1a), (1, p1b)):
        for s in range(2):
            c = 2 * h + s
            nc.tensor.matmul(
                p1t[:, s * CHUNK : (s + 1) * CHUNK],
                tls["w1sb"][:],
                xt[:, c * CHUNK : (c + 1) * CHUNK],
                start=True, stop=True,
            )

    h1a = h1pool.tile([H1, 2 * CHUNK], _BF16, name=f"h1a{q}", tag="h1")
    nc.vector.tensor_scalar(
        h1a[:], p1a[:], tls["b1sb"][:], 0.0,
        mybir.AluOpType.add, mybir.AluOpType.max,
    )
    h1b = h1pool.tile([H1, 2 * CHUNK], _BF16, name=f"h1b{q}", tag="h1")
    if q % 4 == 3:
        nc.vector.tensor_scalar(
            h1b[:], p1b[:], tls["b1sb"][:], 0.0,
            mybir.AluOpType.add, mybir.AluOpType.max,
        )
    else:
        nc.scalar.activation(
            h1b[:], p1b[:], mybir.ActivationFunctionType.Relu,
            bias=tls["b1sb"][:],
        )
    return h1a, h1b


def _quad_tail(nc, q, pools, tls, h1ab, pos, obs):
    """Issue MM2 + tanh + MM3 + (cast + out-DMA) for quad q."""
    (xpool, h1pool, h2pool, obpool, p1pool, p2pool, popool) = pools
    h1a, h1b = h1ab
    p2 = p2pool.tile([128, 2 * CHUNK], _F32, name=f"p2_{q}", tag="p2")
    for h, h1t in ((0, h1a), (1, h1b)):
        for s in range(2):
            nc.tensor.matmul(
                p2[s * H2 : (s + 1) * H2, h * CHUNK : (h + 1) * CHUNK],
                tls["w2sb"][:],
                h1t[:, s * CHUNK : (s + 1) * CHUNK],
                start=True, stop=True,
            )

    h2t = h2pool.tile([128, 2 * CHUNK], _BF16, name=f"h2_{q}", tag="h2")
    nc.scalar.activation(
        h2t[:], p2[:], mybir.ActivationFunctionType.Tanh,
        bias=tls["b2sb"][:],
    )

    # MM3 via block-diag W3blk [128,4]: one matmul per h2 column-pair makes
    # a dense [4,512] output; 4 pairs (2 quads) pack one [*,512] PSUM bank.
    g = q // 2
    if q % 2 == 0:
        pos[g] = popool.tile([128, CHUNK], _F32, name=f"po{g}", tag="po")
    po = pos[g]
    for h in range(2):
        p = 2 * (q % 2) + h
        nc.tensor.matmul(
            po[32 * p : 32 * p + 4, :],
            tls["w3sb"][:],
            h2t[:, h * CHUNK : (h + 1) * CHUNK],
            start=True, stop=True,
            tile_position=(0, 32 * p),
        )

    if q % 2 == 1 or q == NQ - 1:
        piece = g // 4
        if piece not in obs:
            obs[piece] = obpool.tile(
                [128, 4 * CHUNK], _F16, name=f"ob{piece}", tag="ob",
            )
        slot = g % 4
        nc.vector.tensor_copy(
            obs[piece][0:100, slot * CHUNK : (slot + 1) * CHUNK],
            po[0:100, :],
        )
        if g % 4 == 3 or g == NG - 1:
            cols = (slot + 1) * CHUNK
            base = piece * 4 * CHUNK
            for r in range(4):
                nc.sync.dma_start(
                    tls["out_d"][4 * r : 4 * r + 4, base : base + cols],
                    obs[piece][r : r + 97 : 32, 0:cols],
                )


def _build_bass():
    nc = bacc.Bacc(
        "TRN2",
        target_bir_lowering=False,
        debug=False,
        enable_asserts=False,
        num_devices=NCORES,
    )
    x_t = nc.dram_tensor("x_t", [NQ, D, 4 * CHUNK], _BF16, kind="ExternalInput")
    w1 = nc.dram_tensor("w1", [D, H1], _BF16, kind="ExternalInput")
    w2 = nc.dram_tensor("w2", [H1, H2], _BF16, kind="ExternalInput")
    # block-diag [W3 0; 0 W3] (cols: s0c0, s0c1, s1c0, s1c1)
    w3b = nc.dram_tensor("w3b", [128, 4], _BF16, kind="ExternalInput")
    b1 = nc.dram_tensor("b1", [H1, 1], _F32, kind="ExternalInput")
    b2d = nc.dram_tensor("b2d", [128, 1], _F32, kind="ExternalInput")
    # row 4r+p = po partition 32p+r of group g at cols [g*512:(g+1)*512]
    out_d = nc.dram_tensor("out_d", [16, NG * CHUNK], _F16,
                           kind="ExternalOutput")

    with tile.TileContext(nc) as tc:
        with (
            tc.tile_pool(name="w", bufs=1) as wpool,
            tc.tile_pool(name="x", bufs=4) as xpool,
            tc.tile_pool(name="h1", bufs=4) as h1pool,
            tc.tile_pool(name="h2", bufs=3) as h2pool,
            tc.tile_pool(name="ob", bufs=2) as obpool,
            tc.tile_pool(name="p1", bufs=2, space="PSUM") as p1pool,
            tc.tile_pool(name="p2", bufs=1, space="PSUM") as p2pool,
            tc.tile_pool(name="po", bufs=2, space="PSUM") as popool,
        ):
            w1sb = wpool.tile([D, H1], _BF16)
            nc.sync.dma_start(w1sb[:], w1[:, :])
            w2sb = wpool.tile([H1, H2], _BF16)
            nc.sync.dma_start(w2sb[:], w2[:, :])
            w3sb = wpool.tile([128, 4], _BF16)
            nc.sync.dma_start(w3sb[:], w3b[:, :])
            b1sb = wpool.tile([H1, 1], _F32)
            nc.sync.dma_start(b1sb[:], b1[:, :])
            b2sb = wpool.tile([128, 1], _F32)
            nc.sync.dma_start(b2sb[:], b2d[:, :])

            tls = {"x_t": x_t, "out_d": out_d, "w1sb": w1sb, "w2sb": w2sb,
                   "w3sb": w3sb, "b1sb": b1sb, "b2sb": b2sb}
            pools = (xpool, h1pool, h2pool, obpool, p1pool, p2pool, popool)

            pos, obs = {}, {}
            prev = None
            for q in range(NQ):
                h1ab = _quad_head(nc, q, pools, tls)
                if prev is not None:
                    _quad_tail(nc, q - 1, pools, tls, prev, pos, obs)
                prev = h1ab
            _quad_tail(nc, NQ - 1, pools, tls, prev, pos, obs)

    nc.compile()
    return nc


_NC_CACHE = None


def _get_nc():
    global _NC_CACHE
    if _NC_CACHE is None:
        _NC_CACHE = _build_bass()
    return _NC_CACHE


def _F64(x, y):
    return (
        3.0 * (1.0 - x) ** 2 * np.exp(-(x**2) - (y + 1.0) ** 2)
        - 10.0 * (x / 5.0 - x**3 - y**5) * np.exp(-(x**2) - y**2)
        - 1.0 / (3.0 ** np.exp(-((x + 1.0) ** 2) - y**2))
    )


def make_in_maps(data, embed, W1, b1, W2, b2, W3, b3):
    data = np.asarray(data)
    table16 = np.asarray(embed, dtype=np.float32).reshape(-1).astype(
        ml_dtypes.bfloat16)
    W1c = np.ascontiguousarray(np.asarray(W1, np.float32).astype(
        ml_dtypes.bfloat16))
    W2c = np.ascontiguousarray(np.asarray(W2, np.float32).astype(
        ml_dtypes.bfloat16))
    W3c = np.asarray(W3, np.float32)
    w3blk = np.zeros((128, 4), np.float32)
    w3blk[0:64, 0:2] = W3c
    w3blk[64:128, 2:4] = W3c
    w3blk = np.ascontiguousarray(w3blk.astype(ml_dtypes.bfloat16))
    b1c = np.ascontiguousarray(b1, dtype=np.float32).reshape(H1, 1)
    b2c = np.asarray(b2, dtype=np.float32).reshape(H2, 1)
    b2d = np.concatenate([b2c, b2c], axis=0)          # [128, 1]

    in_maps = []
    for c in range(NCORES):
        dshard = data[c * NPC_RAW : (c + 1) * NPC_RAW]
        dpad = np.zeros((NPC, D), dtype=dshard.dtype)
        dpad[:NPC_RAW] = dshard
        xt = np.ascontiguousarray(
            table16[dpad.reshape(NQ, 4 * CHUNK, D).transpose(0, 2, 1)]
        )
        in_maps.append(
            {"x_t": xt, "w1": W1c, "w2": W2c, "w3b": w3blk,
             "b1": b1c, "b2d": b2d}
        )
    return in_maps


def _decode_outs(res):
    """-> out0_all, out1_all fp32 arrays of shape [N] (padding stripped)."""
    o0s, o1s = [], []
    for c in range(NCORES):
        od = np.asarray(res.results[c]["out_d"], np.float32)
        arr = od.reshape(4, 4, NG, CHUNK)           # [r, p, g, i]
        o0 = np.empty((CHUNKS, CHUNK), np.float32)
        o1 = np.empty((CHUNKS, CHUNK), np.float32)
        for r in range(4):
            for p in range(4):
                ch = 4 * (p // 2) + 2 * (p % 2) + (r // 2)  # chunk-in-group
                dst = o0 if r % 2 == 0 else o1
                ks = np.arange(NG) * 8 + ch
                valid = ks < CHUNKS
                dst[ks[valid]] = arr[r, p, valid]
        o0s.append(o0.reshape(-1)[:NPC_RAW])
        o1s.append(o1.reshape(-1)[:NPC_RAW])
    return np.concatenate(o0s), np.concatenate(o1s)


def kernel(data, embed, W1, b1, W2, b2, W3, b3):
    data = np.asarray(data)
    nc = _get_nc()
    in_maps = make_in_maps(data, embed, W1, b1, W2, b2, W3, b3)
    res = run_bass_kernel_spmd(nc, in_maps, core_ids=list(range(NCORES)))
    o0, o1 = _decode_outs(res)

    pred = _F64(o0.astype(np.float64), o1.astype(np.float64))
    K = 4096
    cand = np.argpartition(pred, N - K)[N - K:]

    table32 = np.asarray(embed, dtype=np.float32).reshape(-1)
    W1f = np.asarray(W1, np.float32)
    W2f = np.asarray(W2, np.float32)
    W3f = np.asarray(W3, np.float32)
    xk = table32[data[cand]]
    hk = np.maximum(xk @ W1f + np.asarray(b1, np.float32), 0.0)
    hk = np.tanh(hk @ W2f + np.asarray(b2, np.float32))
    ok = hk @ W3f + np.asarray(b3, np.float32)
    pk = _F64(ok[:, 0].astype(np.float64), ok[:, 1].astype(np.float64))
    return ok[int(np.argmax(pk))].astype(np.float32)


# revision 13
# speedup vs baseline: 1.1757x; 1.0324x over previous
"""Trainium2 Bass kernel for nn_NeuralNet_19516331393457 (dense_mlp).

Pipeline: x = embed[data] (48-entry table); h1 = relu(x@W1+b1);
h2 = tanh(h1@W2+b2); out = h2@W3+b3; return out[argmax(F(out0, out1))].

v3 strategy (data-parallel over N=500000 on 8 cores, bf16 device math):
  - Host: tiny-table gather embed[data] in bf16, tile-blocked transpose to
    [NQ, 128, 2048] per core (quads of 4 chunks x 512 samples).
  - Device, software-pipelined per quad q: MM1(q) x4 -> 2x [128,1024] PSUM;
    relu(q) split DVE/ACT -> h1 bf16; then the *previous* quad's tail:
    MM2(q-1) x4 column-packed 2-up into one [128,1024] PSUM; one ACT tanh
    evicts 4 chunks; MM3 via block-diag W3blk=[W3 0; 0 W3] -> dense [4,512]
    outputs at partition 32p of a per-2-quad po bank; one DVE cast per 2
    quads -> fp16 staging; strided output DMAs per 8-quad piece.
  - Host: decode outs, F in fp64, exact top-K rescore in fp32 (device
    ordering only needs the winner inside the top-K shortlist).
"""

import numpy as np
import ml_dtypes

import concourse.mybir as mybir
import concourse.tile as tile
from concourse import bacc
from concourse.bass_utils import run_bass_kernel_spmd

N = 500000
D = 128
H1 = 128
H2 = 64
NCLS = 2
NCORES = 8
CHUNK = 512
NPC_RAW = N // NCORES              # 62500 samples per core
NQ = 31                            # quads per core (4 chunks each)
CHUNKS = 4 * NQ                    # 124
NPC = CHUNKS * CHUNK               # 63488 padded samples per core
NG = 16                            # 2-quad groups (last is half)
PIECE = 8                          # quads per output staging piece
NPIECE = 4

_F32 = mybir.dt.float32
_F16 = mybir.dt.float16
_BF16 = mybir.dt.bfloat16


def _issue_x_dma(nc, q, pools, tls, xts):
    (xpool, h1pool, h2pool, obs_pool, p1pool, p2pool, popool) = pools
    xt = xpool.tile([D, 4 * CHUNK], _BF16, name=f"xt{q}", tag="xt")
    nc.sync.dma_start(xt[:], tls["x_t"][q, :, :])
    xts[q] = xt


def _quad_head(nc, q, pools, tls, xts):
    """Issue MM1 + relu for quad q (x DMA pre-issued); prefetch x of q+2."""
    (xpool, h1pool, h2pool, obs_pool, p1pool, p2pool, popool) = pools
    if q + 2 < NQ:
        _issue_x_dma(nc, q + 2, pools, tls, xts)
    xt = xts.pop(q)

    p1a = p1pool.tile([H1, 2 * CHUNK], _F32, name=f"p1a{q}", tag="p1")
    p1b = p1pool.tile([H1, 2 * CHUNK], _F32, name=f"p1b{q}", tag="p1")
    for h, p1t in ((0, p1a), (1, p1b)):
        for s in range(2):
            c = 2 * h + s
            nc.tensor.matmul(
                p1t[:, s * CHUNK : (s + 1) * CHUNK],
                tls["w1sb"],
                xt[:, c * CHUNK : (c + 1) * CHUNK],
                start=True, stop=True,
            )

    h1a = h1pool.tile([H1, 2 * CHUNK], _BF16, name=f"h1a{q}", tag="h1")
    nc.vector.tensor_scalar(
        h1a[:], p1a[:], tls["b1sb"], 0.0,
        mybir.AluOpType.add, mybir.AluOpType.max,
    )
    h1b = h1pool.tile([H1, 2 * CHUNK], _BF16, name=f"h1b{q}", tag="h1")
    if q % 4 == 3:
        nc.vector.tensor_scalar(
            h1b[:], p1b[:], tls["b1sb"], 0.0,
            mybir.AluOpType.add, mybir.AluOpType.max,
        )
    else:
        nc.scalar.activation(
            h1b[:], p1b[:], mybir.ActivationFunctionType.Relu,
            bias=tls["b1sb"],
        )
    return h1a, h1b


def _quad_tail(nc, q, pools, tls, h1ab, pos, obs):
    """Issue MM2 + tanh + MM3 + (cast + out-DMA) for quad q."""
    (xpool, h1pool, h2pool, obpool, p1pool, p2pool, popool) = pools
    h1a, h1b = h1ab
    p2 = p2pool.tile([128, 2 * CHUNK], _F32, name=f"p2_{q}", tag="p2")
    for h, h1t in ((0, h1a), (1, h1b)):
        for s in range(2):
            nc.tensor.matmul(
                p2[s * H2 : (s + 1) * H2, h * CHUNK : (h + 1) * CHUNK],
                tls["w2sb"],
                h1t[:, s * CHUNK : (s + 1) * CHUNK],
                start=True, stop=True,
            )

    h2t = h2pool.tile([128, 2 * CHUNK], _BF16, name=f"h2_{q}", tag="h2")
    nc.scalar.activation(
        h2t[:], p2[:], mybir.ActivationFunctionType.Tanh,
        bias=tls["b2sb"],
    )

    # MM3 via block-diag W3blk [128,4]: one matmul per h2 column-pair makes
    # a dense [4,512] output; 4 pairs (2 quads) pack one [*,512] PSUM bank.
    g = q // 2
    if q % 2 == 0:
        pos[g] = popool.tile([128, CHUNK], _F32, name=f"po{g}", tag="po")
    po = pos[g]
    for h in range(2):
        p = 2 * (q % 2) + h
        nc.tensor.matmul(
            po[32 * p : 32 * p + 4, :],
            tls["w3sb"],
            h2t[:, h * CHUNK : (h + 1) * CHUNK],
            start=True, stop=True,
            tile_position=(0, 32 * p),
        )

    if q % 2 == 1 or q == NQ - 1:
        piece = g // 4
        if piece not in obs:
            obs[piece] = obpool.tile(
                [128, 4 * CHUNK], _F16, name=f"ob{piece}", tag="ob",
            )
        slot = g % 4
        nc.vector.tensor_copy(
            obs[piece][0:100, slot * CHUNK : (slot + 1) * CHUNK],
            po[0:100, :],
        )
        if g % 4 == 3 or g == NG - 1:
            cols = (slot + 1) * CHUNK
            base = piece * 4 * CHUNK
            for r in range(4):
                nc.sync.dma_start(
                    tls["out_d"][4 * r : 4 * r + 4, base : base + cols],
                    obs[piece][r : r + 97 : 32, 0:cols],
                )


def _build_bass():
    nc = bacc.Bacc(
        "TRN2",
        target_bir_lowering=False,
        debug=False,
        enable_asserts=False,
        num_devices=NCORES,
    )
    x_t = nc.dram_tensor("x_t", [NQ, D, 4 * CHUNK], _BF16, kind="ExternalInput")
    # packed weights: cols [0:128]=W1, [128:192]=W2, [192:196]=blockdiag W3
    wpk = nc.dram_tensor("wpk", [128, H1 + H2 + 4], _BF16,
                         kind="ExternalInput")
    # packed biases: col 0 = b1, col 1 = [b2; b2]
    bpk = nc.dram_tensor("bpk", [128, 2], _F32, kind="ExternalInput")
    # row 4r+p = po partition 32p+r of group g at cols [g*512:(g+1)*512]
    out_d = nc.dram_tensor("out_d", [16, NG * CHUNK], _F16,
                           kind="ExternalOutput")

    with tile.TileContext(nc) as tc:
        with (
            tc.tile_pool(name="w", bufs=1) as wpool,
            tc.tile_pool(name="x", bufs=4) as xpool,
            tc.tile_pool(name="h1", bufs=4) as h1pool,
            tc.tile_pool(name="h2", bufs=3) as h2pool,
            tc.tile_pool(name="ob", bufs=2) as obpool,
            tc.tile_pool(name="p1", bufs=2, space="PSUM") as p1pool,
            tc.tile_pool(name="p2", bufs=1, space="PSUM") as p2pool,
            tc.tile_pool(name="po", bufs=2, space="PSUM") as popool,
        ):
            tls = {"x_t": x_t, "out_d": out_d}
            pools = (xpool, h1pool, h2pool, obpool, p1pool, p2pool, popool)
            xts = {}

            # x tiles of the first two quads first, so MM1(0) starts ASAP;
            # weights packed as two DMA issues right behind them
            _issue_x_dma(nc, 0, pools, tls, xts)
            _issue_x_dma(nc, 1, pools, tls, xts)
            wsb = wpool.tile([128, H1 + H2 + 4], _BF16)
            nc.sync.dma_start(wsb[:], wpk[:, :])
            bsb = wpool.tile([128, 2], _F32)
            nc.sync.dma_start(bsb[:], bpk[:, :])
            tls.update({
                "w1sb": wsb[:, 0:H1], "w2sb": wsb[:, H1 : H1 + H2],
                "w3sb": wsb[:, H1 + H2 : H1 + H2 + 4],
                "b1sb": bsb[:, 0:1], "b2sb": bsb[:, 1:2],
            })

            pos, obs = {}, {}
            prev = None
            for q in range(NQ):
                h1ab = _quad_head(nc, q, pools, tls, xts)
                if prev is not None:
                    _quad_tail(nc, q - 1, pools, tls, prev, pos, obs)
                prev = h1ab
            _quad_tail(nc, NQ - 1, pools, tls, prev, pos, obs)

    nc.compile()
    return nc


_NC_CACHE = None


def _get_nc():
    global _NC_CACHE
    if _NC_CACHE is None:
        _NC_CACHE = _build_bass()
    return _NC_CACHE


def _F64(x, y):
    return (
        3.0 * (1.0 - x) ** 2 * np.exp(-(x**2) - (y + 1.0) ** 2)
        - 10.0 * (x / 5.0 - x**3 - y**5) * np.exp(-(x**2) - y**2)
        - 1.0 / (3.0 ** np.exp(-((x + 1.0) ** 2) - y**2))
    )


def make_in_maps(data, embed, W1, b1, W2, b2, W3, b3):
    data = np.asarray(data)
    table16 = np.asarray(embed, dtype=np.float32).reshape(-1).astype(
        ml_dtypes.bfloat16)
    wpk = np.zeros((128, H1 + H2 + 4), np.float32)
    wpk[:, 0:H1] = np.asarray(W1, np.float32)
    wpk[0:64, H1 : H1 + H2] = np.asarray(W2, np.float32)[0:64]
    wpk[64:128, H1 : H1 + H2] = np.asarray(W2, np.float32)[64:128]
    W3c = np.asarray(W3, np.float32)
    wpk[0:64, H1 + H2 : H1 + H2 + 2] = W3c
    wpk[64:128, H1 + H2 + 2 : H1 + H2 + 4] = W3c
    wpk = np.ascontiguousarray(wpk.astype(ml_dtypes.bfloat16))
    b2c = np.asarray(b2, dtype=np.float32).reshape(H2, 1)
    bpk = np.zeros((128, 2), np.float32)
    bpk[:, 0:1] = np.ascontiguousarray(b1, dtype=np.float32).reshape(H1, 1)
    bpk[:, 1:2] = np.concatenate([b2c, b2c], axis=0)

    in_maps = []
    for c in range(NCORES):
        dshard = data[c * NPC_RAW : (c + 1) * NPC_RAW]
        dpad = np.zeros((NPC, D), dtype=dshard.dtype)
        dpad[:NPC_RAW] = dshard
        xt = np.ascontiguousarray(
            table16[dpad.reshape(NQ, 4 * CHUNK, D).transpose(0, 2, 1)]
        )
        in_maps.append({"x_t": xt, "wpk": wpk, "bpk": bpk})
    return in_maps


def _decode_outs(res):
    """-> out0_all, out1_all fp32 arrays of shape [N] (padding stripped)."""
    o0s, o1s = [], []
    for c in range(NCORES):
        od = np.asarray(res.results[c]["out_d"], np.float32)
        arr = od.reshape(4, 4, NG, CHUNK)           # [r, p, g, i]
        o0 = np.empty((CHUNKS, CHUNK), np.float32)
        o1 = np.empty((CHUNKS, CHUNK), np.float32)
        for r in range(4):
            for p in range(4):
                ch = 4 * (p // 2) + 2 * (p % 2) + (r // 2)  # chunk-in-group
                dst = o0 if r % 2 == 0 else o1
                ks = np.arange(NG) * 8 + ch
                valid = ks < CHUNKS
                dst[ks[valid]] = arr[r, p, valid]
        o0s.append(o0.reshape(-1)[:NPC_RAW])
        o1s.append(o1.reshape(-1)[:NPC_RAW])
    return np.concatenate(o0s), np.concatenate(o1s)


def kernel(data, embed, W1, b1, W2, b2, W3, b3):
    data = np.asarray(data)
    nc = _get_nc()
    in_maps = make_in_maps(data, embed, W1, b1, W2, b2, W3, b3)
    res = run_bass_kernel_spmd(nc, in_maps, core_ids=list(range(NCORES)))
    o0, o1 = _decode_outs(res)

    pred = _F64(o0.astype(np.float64), o1.astype(np.float64))
    K = 4096
    cand = np.argpartition(pred, N - K)[N - K:]

    table32 = np.asarray(embed, dtype=np.float32).reshape(-1)
    W1f = np.asarray(W1, np.float32)
    W2f = np.asarray(W2, np.float32)
    W3f = np.asarray(W3, np.float32)
    xk = table32[data[cand]]
    hk = np.maximum(xk @ W1f + np.asarray(b1, np.float32), 0.0)
    hk = np.tanh(hk @ W2f + np.asarray(b2, np.float32))
    ok = hk @ W3f + np.asarray(b3, np.float32)
    pk = _F64(ok[:, 0].astype(np.float64), ok[:, 1].astype(np.float64))
    return ok[int(np.argmax(pk))].astype(np.float32)
